# revision 5
# baseline (speedup 1.0000x reference)
"""Trainium2 Bass kernel for dynamic low-pass filter decomposition, v5.

Module: global-avg-pool -> 1x1 conv -> BN (inference) -> softmax over 3x3
taps gives a per-(sample, group) 3x3 kernel; applied as a reflect-padded
depthwise conv over x; returns (low, x - low).

Sharding: data-parallel over batch n=8 across 8 NeuronCores (1 sample/core).

v5 changes vs v4 (v4 = 87us, load phase issue-bound + cold PE):
  - x loads coalesced: 14 per-band flat descriptors (DMA_DIRECT2D issue
    costs ~650ns/inst on the engine; v4's 42 descriptors serialized the
    load phase) + 2 coalesced halo descriptors + 2 edge rows.
  - pooled sums: one DVE tensor_reduce per band (f32 out) + tiny gpsimd
    accumulate across bands; ONE pooled matmul at the end.  The PE queue
    during the load phase runs back-to-back dummy matmuls so the HAM
    activity monitor ramps the PE to 2.4 GHz before the conv starts
    (v4 ran the conv at 1.2 GHz for its first 10us).
  - weight-gen chain in bf16 (single-pass matmuls instead of fp32
    LOW_HIGH double passes).
  - conv phase: PSUM->SBUF low copy split scalar/DVE halves, edge
    scatter split, low store on scalar HW queue + high store on sync HW
    queue (v4 put low on gpsimd's software-DGE queue).

Row-band layout (unchanged from v4): 192 rows in 14 bands of 14 rows;
partition p = i*8 + g (i row-in-band, g channel group); halo rows at
partitions 112..119 (above) / 120..127 (below); free dim = 8 c_sub x
192 w = 1536 per partition with 2-elem zero pads per band window.
3 block-banded stationaries S_dj fold the vertical taps in-array.
"""
import sys
import os

sys.path.insert(0, "/opt/trn_rl_repo")

import numpy as np
import ml_dtypes
from contextlib import ExitStack

import concourse.bass as bass
import concourse.tile as tile
from concourse import bacc, mybir
from concourse.bass_utils import run_bass_kernel_spmd

dt = mybir.dt
f32 = dt.float32
bf16 = dt.bfloat16

KS = 3
GROUP = 8
IC = 64
BN_EPS = 1e-5
N = 8
H = W = 192
CW = 8 * W              # free elems per partition (8 chans x 192 cols)
PAD = 2                 # front/back pad elems (4B alignment + shift room)
BR = 14                 # rows per band
NBANDS = 14             # 14 * 14 = 196 >= 192
CH = 512
ROWSTRIDE = IC * W      # 12288 elems per image row in [r][c][w] layout
BW = PAD + CW + PAD     # band window in SBUF free dim
TSTRIDE = BR * ROWSTRIDE  # 172032 elems per band of DRAM rows

DUMMY_N = 42            # PE warm-up matmuls during the load phase


def _band_rows(t):
    """(first output row, n output rows) of band t."""
    r0 = BR * t
    return r0, min(BR, H - r0)


def _build_program():
    nc = bacc.Bacc("TRN2", target_bir_lowering=False, debug=False,
                   num_devices=N)

    x_d = nc.dram_tensor("x", [H, IC, W], bf16, kind="ExternalInput")
    e_d = [nc.dram_tensor(f"epat{di}", [128, 128], bf16,
                          kind="ExternalInput") for di in range(3)]
    hv_d = nc.dram_tensor("hv4", [8, 128], bf16, kind="ExternalInput")
    gm_d = nc.dram_tensor("gmask", [128, 8], f32, kind="ExternalInput")
    a8_d = nc.dram_tensor("a8", [8, 576], bf16, kind="ExternalInput")
    b_d = nc.dram_tensor("b72", [72, 1], f32, kind="ExternalInput")
    r9_d = nc.dram_tensor("r9", [72, 9], bf16, kind="ExternalInput")
    g_d = nc.dram_tensor("g728", [72, 8], bf16, kind="ExternalInput")
    low_d = nc.dram_tensor("low", [H, IC, W], bf16, kind="ExternalOutput")
    high_d = nc.dram_tensor("high", [H, IC, W], bf16, kind="ExternalOutput")

    xd = x_d.ap().tensor

    def band_main_ap(t):
        """DRAM AP for band t's valid rows: partition (i, g) = i*8+g <-
        row r0+i, chans 8g..8g+8.  Since ROWSTRIDE = 8*CW this is a FLAT
        2-dim AP (partition stride CW) -- one contiguous 344KB burst."""
        r0, nr = _band_rows(t)
        return bass.AP(xd, r0 * ROWSTRIDE,
                       [[ROWSTRIDE, nr], [CW, 8], [1, CW]])

    def row_ap(r):
        """DRAM AP for one image row across the 8 group partitions."""
        return bass.AP(xd, r * ROWSTRIDE, [[CW, 8], [1, CW]])

    def out_ap(dram, t):
        r0, nr = _band_rows(t)
        return bass.AP(dram.ap().tensor, r0 * ROWSTRIDE,
                       [[ROWSTRIDE, nr], [CW, 8], [1, CW]])

    with tile.TileContext(nc) as tc, ExitStack() as ctx:
        cpool = ctx.enter_context(tc.tile_pool(name="consts", bufs=1))
        xpool = ctx.enter_context(tc.tile_pool(name="x", bufs=1))
        wpool = ctx.enter_context(tc.tile_pool(name="w", bufs=1))
        rpool = ctx.enter_context(tc.tile_pool(name="red", bufs=2))
        spool = ctx.enter_context(tc.tile_pool(name="stage", bufs=3))

        # ---- PE warm-up: dummy matmuls on memset tiles keep the PE's
        # HAM activity window busy through the load phase so the conv
        # starts at 2.4 GHz (K=8/8) instead of 1.2 ----
        warm_s = wpool.tile([128, 128], bf16, name="warm_s")
        warm_m = wpool.tile([128, 512], bf16, name="warm_m")
        nc.vector.memset(warm_s[:], 0.0)
        nc.vector.memset(warm_m[:], 0.0)
        wps_cm = tc.tile_pool(name="wpsum", bufs=1,
                              space=bass.MemorySpace.PSUM)
        wps = wps_cm.__enter__()
        warm_p = wps.tile([128, 512], f32, tag="warm")
        for _ in range(DUMMY_N):
            nc.tensor.matmul(warm_p[:], warm_s[:], warm_m[:],
                             start=True, stop=True)

        # ---- band loads: one flat descriptor per band (sync/scalar
        # alternating), then 2 coalesced halo descriptors + edge rows --
        xball = xpool.tile([128, NBANDS * BW], bf16)
        xb = [xball[:, t * BW:(t + 1) * BW] for t in range(NBANDS)]
        xv = xball[:].rearrange("p (t b) -> p t b", b=BW)
        # band 13 has unloaded partition rows; zero them (32-aligned
        # base) BEFORE its loads so the overlapping DMAs order after
        # the memset.  NaN in any streamed partition would poison the
        # conv accumulator column (0 * NaN = NaN in the PE array).
        nc.gpsimd.memset(xb[NBANDS - 1][64:128, :], 0.0)
        for t in range(NBANDS):
            r0, nr = _band_rows(t)
            eng = nc.sync if t % 2 == 0 else nc.scalar
            eng.dma_start(xb[t][0:8 * nr, PAD:PAD + CW], band_main_ap(t))
        # halo rows above, bands 1..13 in one descriptor: partition
        # 112+g <- row 14t-1 chans 8g..; src strides [g 1536][t 172032]
        nc.sync.dma_start(
            xv[112:120, 1:NBANDS, PAD:PAD + CW],
            bass.AP(xd, (BR - 1) * ROWSTRIDE,
                    [[CW, 8], [TSTRIDE, NBANDS - 1], [1, CW]]))
        # halo rows below, bands 0..12 in one descriptor
        nc.scalar.dma_start(
            xv[120:128, 0:NBANDS - 1, PAD:PAD + CW],
            bass.AP(xd, BR * ROWSTRIDE,
                    [[CW, 8], [TSTRIDE, NBANDS - 1], [1, CW]]))
        # top edge reflect (band 0 above <- row 1); bottom edge reflect
        # (band 13 below <- row 190 at partitions 8*nr = 80)
        nc.sync.dma_start(xb[0][112:120, PAD:PAD + CW], row_ap(1))
        _, nr13 = _band_rows(NBANDS - 1)
        nc.scalar.dma_start(xb[NBANDS - 1][8 * nr13:8 * nr13 + 8,
                                           PAD:PAD + CW], row_ap(H - 2))

        # ---- consts (gpsimd software queue; all tiny) ----
        e_s = [cpool.tile([128, 128], bf16, name=f"epat{di}")
               for di in range(3)]
        hv_s = cpool.tile([8, 128], bf16)
        gm_s = cpool.tile([128, 8], f32)
        a8_s = cpool.tile([8, 576], bf16)
        b_s = cpool.tile([72, 1], f32)
        r9_s = cpool.tile([72, 9], bf16)
        g_s = cpool.tile([72, 8], bf16)
        for s, d in ((b_s, b_d), (gm_s, gm_d), (a8_s, a8_d),
                     (e_s[0], e_d[0]), (e_s[1], e_d[1]), (e_s[2], e_d[2]),
                     (hv_s, hv_d), (r9_s, r9_d), (g_s, g_d)):
            nc.gpsimd.dma_start(s[:], d.ap())

        # ---- zero fills: band-window pads (2 strided memsets cover all
        # bands) + band 13's never-loaded partitions.  The conv matmuls
        # stream all 128 partitions; a NaN anywhere would poison the
        # whole accumulator column (0 * NaN = NaN in the PE array). ----
        nc.vector.memset(xv[:, :, 0:PAD], 0.0)
        nc.vector.memset(xv[:, :, PAD + CW:BW], 0.0)

        # pre-load ACT spline tables off the weight-chain critical path
        exp_dummy = wpool.tile([72, 1], f32)
        nc.scalar.activation(exp_dummy[:], b_s[:],
                             mybir.ActivationFunctionType.Exp)

        # ---- pooled sums: one DVE reduce per band -> [np, 8] f32;
        # gpsimd accumulates across bands into acc8 ----
        acc8 = wpool.tile([128, 8], f32, name="acc8")
        nc.gpsimd.memset(acc8[:], 0.0)
        for t in range(NBANDS):
            _, nr = _band_rows(t)
            np_ = 8 * nr
            bs_t = rpool.tile([128, 8], f32, tag="bs", name=f"bs{t}")
            nc.vector.tensor_reduce(
                bs_t[0:np_, :].rearrange("p (c o) -> p c o", o=1),
                xb[t][0:np_, PAD:PAD + CW].rearrange("p (c w) -> p c w",
                                                     w=W),
                axis=mybir.AxisListType.X, op=mybir.AluOpType.add)
            nc.gpsimd.tensor_tensor(acc8[0:np_, :], acc8[0:np_, :],
                                    bs_t[0:np_, :],
                                    op=mybir.AluOpType.add)

        # ---- weight generation chain ----
        # P64[g, cs] = per-(group, chan-sub) pooled sum
        p64_p = wps.tile([8, 8], f32, tag="p64")
        nc.tensor.matmul(p64_p[:], gm_s[0:112, :], acc8[0:112, :])
        p64s = wpool.tile([8, 8], bf16)
        nc.scalar.copy(p64s[:], p64_p[:])
        # logits: lf[oc] = sum_cs sum_g a8[g, 72*cs+oc] * P64[g, cs]
        lf_p = wps.tile([72, 1], f32, tag="lf")
        for cs in range(8):
            nc.tensor.matmul(lf_p[:], a8_s[:, 72 * cs:72 * (cs + 1)],
                             p64s[:, cs:cs + 1],
                             start=(cs == 0), stop=(cs == 7))
        e72 = wpool.tile([72, 1], f32)
        nc.scalar.activation(e72[:], lf_p[:],
                             mybir.ActivationFunctionType.Exp,
                             bias=b_s[:, 0:1], scale=1.0)
        rhsw = wpool.tile([72, 9], bf16)
        nc.vector.tensor_scalar_mul(rhsw[:], r9_s[:], e72[:, 0:1])
        w89_p = wps.tile([8, 9], f32, tag="w89")
        nc.tensor.matmul(w89_p[:], g_s[:], rhsw[:])
        s8 = wpool.tile([8, 1], f32)
        nc.vector.tensor_reduce(s8[:], w89_p[:],
                                axis=mybir.AxisListType.X,
                                op=mybir.AluOpType.add)
        r8 = wpool.tile([8, 1], f32)
        nc.vector.reciprocal(r8[:], s8[:])
        w89s = wpool.tile([8, 9], bf16)
        nc.vector.tensor_scalar_mul(w89s[:], w89_p[:], r8[:, 0:1])
        wbig_p = wps.tile([128, 9], f32, tag="wbig")
        nc.tensor.matmul(wbig_p[:], hv_s[:], w89s[:])
        wsc = wpool.tile([128, 9], f32)
        nc.scalar.copy(wsc[:], wbig_p[:])
        wps_cm.__exit__(None, None, None)

        # ---- the 3 block-banded stationaries: S_dj = sum_di E_di *
        # w[g(q), 3*di+dj] (per-partition row scaling; g(q) = q%8) ----
        S = [wpool.tile([128, 128], bf16, name=f"S{dj}") for dj in range(3)]
        for dj in range(3):
            nc.vector.tensor_scalar_mul(S[dj][:], e_s[0][:],
                                        wsc[:, dj:dj + 1])
            for di in (1, 2):
                nc.vector.scalar_tensor_tensor(
                    S[dj][:], e_s[di][:], wsc[:, 3 * di + dj:3 * di + dj + 1],
                    S[dj][:],
                    op0=mybir.AluOpType.mult, op1=mybir.AluOpType.add)

        # ---- main loop: one band at a time, acc = 4 PSUM banks
        # (3 x 512 main + edge-fix columns in bank 3) ----
        mpool = ctx.enter_context(
            tc.tile_pool(name="mpsum", bufs=2, space=bass.MemorySpace.PSUM))
        HALF = CW // 2
        for t in range(NBANDS):
            _, nr = _band_rows(t)
            np_ = 8 * nr
            acc = mpool.tile([128, 2048], f32, tag="acc", name=f"acc{t}")
            djs = (0, 1, 2) if t % 2 == 0 else (2, 1, 0)
            for j, dj in enumerate(djs):
                first, last = (j == 0), (j == 2)
                for ch in range(3):
                    off = PAD + CH * ch + dj - 1
                    nc.tensor.matmul(acc[:, CH * ch:CH * (ch + 1)],
                                     S[dj][:], xb[t][:, off:off + CH],
                                     start=first, stop=last)
                wl = (1, 0, 1)[dj]
                wr = (190, 191, 190)[dj]
                ev = xb[t][:, PAD:PAD + CW].rearrange(
                    "p (c w) -> p c w", w=W)[:, :, wl:wr + 1:wr - wl]
                nc.tensor.matmul(
                    acc[:, 1536:1552].rearrange("p (c e) -> p c e", e=2),
                    S[dj][:], ev, start=first, stop=last)
            low_st = spool.tile([128, CW], bf16, tag="low")
            # PSUM->SBUF drain split across scalar and DVE; edge columns
            # (reflect at w=0/191, correct values in acc bank 3) split too
            nc.scalar.copy(low_st[:, 0:HALF], acc[:, 0:HALF])
            nc.vector.tensor_scalar_mul(low_st[:, HALF:CW],
                                        acc[:, HALF:CW], 1.0)
            lv = low_st[:].rearrange("p (c w) -> p c w", w=W)
            av = acc[:, 1536:1552].rearrange("p (c e) -> p c e", e=2)
            nc.scalar.copy(lv[:, 0:4, 0:W:W - 1], av[:, 0:4, :])
            nc.vector.tensor_scalar_mul(lv[:, 4:8, 0:W:W - 1],
                                        av[:, 4:8, :], 1.0)
            high_st = spool.tile([128, CW], bf16, tag="high")
            nc.vector.tensor_tensor(high_st[0:np_, :],
                                    xb[t][0:np_, PAD:PAD + CW],
                                    low_st[0:np_, :],
                                    op=mybir.AluOpType.subtract)
            nc.scalar.dma_start(out_ap(low_d, t), low_st[0:np_, :])
            nc.sync.dma_start(out_ap(high_d, t), high_st[0:np_, :])

    nc.compile()
    return nc


_nc_cache = None


def _get_program():
    global _nc_cache
    if _nc_cache is None:
        _nc_cache = _build_program()
    return _nc_cache


def _host_consts(conv_w, bn_gamma, bn_beta, bn_mean, bn_var):
    s_a = bn_gamma / np.sqrt(bn_var + BN_EPS)
    b72 = (bn_beta - bn_mean * s_a).astype(np.float32).reshape(72, 1)
    A = (conv_w * s_a[:, None]) / np.float32(H * W)   # (72, 64)

    # E wiring patterns: epat[di][q, p] = 1 iff q is the source partition
    # of output partition p for vertical tap di (halo rows at 112/120)
    epat = [np.zeros((128, 128), np.float32) for _ in range(3)]
    for p in range(128):
        i_out, g = p // 8, p % 8
        for di in range(3):
            i_src = i_out + di - 1
            if i_src == -1:
                q = 112 + g
            elif i_src == BR:
                q = 120 + g
            elif 0 <= i_src < BR:
                q = i_src * 8 + g
            else:
                continue
            epat[di][q, p] = 1.0

    epat = [e.astype(ml_dtypes.bfloat16) for e in epat]
    hv4 = (np.arange(8)[:, None] == (np.arange(128)[None, :] % 8)
           ).astype(ml_dtypes.bfloat16)
    gmask = ((np.arange(128)[:, None] % 8) == np.arange(8)[None, :]
             ).astype(np.float32)
    # a8[g, 72*cs + oc] = A[oc, 8g + cs]
    a8 = np.zeros((8, 576), np.float32)
    for g in range(8):
        for cs in range(8):
            a8[g, 72 * cs:72 * (cs + 1)] = A[:, 8 * g + cs]
    a8 = a8.astype(ml_dtypes.bfloat16)
    oc = np.arange(72)
    r9 = (oc[:, None] % 9 == np.arange(9)[None, :]
          ).astype(ml_dtypes.bfloat16)
    g728 = (oc[:, None] // 9 == np.arange(8)[None, :]
            ).astype(ml_dtypes.bfloat16)
    return dict(epat0=epat[0], epat1=epat[1], epat2=epat[2], hv4=hv4,
                gmask=gmask, a8=a8, b72=b72, r9=r9, g728=g728)


def _prep_inputs(x, conv_w, bn_gamma, bn_beta, bn_mean, bn_var):
    x = np.asarray(x, np.float32)
    consts = _host_consts(np.asarray(conv_w, np.float32),
                          np.asarray(bn_gamma, np.float32),
                          np.asarray(bn_beta, np.float32),
                          np.asarray(bn_mean, np.float32),
                          np.asarray(bn_var, np.float32))
    maps = []
    for i in range(N):
        xr = np.ascontiguousarray(np.transpose(x[i], (1, 0, 2))
                                  ).astype(ml_dtypes.bfloat16)
        maps.append(dict(x=xr, **consts))
    return maps


def _gather(res):
    low = np.stack([np.transpose(np.asarray(res[i]["low"]), (1, 0, 2))
                    for i in range(N)]).astype(np.float32)
    high = np.stack([np.transpose(np.asarray(res[i]["high"]), (1, 0, 2))
                     for i in range(N)]).astype(np.float32)
    return low, high


def kernel(x, conv_w, bn_gamma, bn_beta, bn_mean, bn_var):
    in_maps = _prep_inputs(x, conv_w, bn_gamma, bn_beta, bn_mean, bn_var)
    nc = _get_program()
    res = run_bass_kernel_spmd(nc, in_maps, list(range(N))).results
    return _gather(res)


if __name__ == "__main__":
    rng = np.random.default_rng(0)
    demo = dict(
        x=rng.standard_normal((N, IC, H, W), dtype=np.float32),
        conv_w=rng.standard_normal((72, 64)).astype(np.float32),
        bn_gamma=np.ones(72, np.float32),
        bn_beta=np.zeros(72, np.float32),
        bn_mean=rng.standard_normal(72).astype(np.float32) * 0.1,
        bn_var=rng.uniform(0.5, 1.5, 72).astype(np.float32),
    )
    low, high = kernel(**demo)
    print("ok", low.shape, high.shape)
